# revision 1
# baseline (speedup 1.0000x reference)
"""DigitCaps dynamic-routing kernel for 8 Trainium2 NeuronCores.

Problem: x(32,16384,8) f32, W(10,16384,8,16) f32 -> v(32,10,16) f32
  u_hat[b,j,p,o] = sum_d x[b,p,d] W[j,p,d,o]   (never materialized!)
  3 routing iterations (softmax over j, weighted sums over p).

Strategy: shard P=16384 over 8 cores (P_loc=2048). Per routing iteration:
  s_part[b,j,o] = sum_{p,d} (c*x)[b,j,p,d] * W[j,p,d,o]     (PE, K=p 128-chunks)
  AllReduce s (20KB) -> v = squash(s)
  z[b,j,p,d]  = sum_o W[j,p,d,o] v[b,j,o]                   (PE, K=(d,o)=128 with
                                                             block-diagonal v rhs)
  uv[b,j,p]   = sum_d x[b,p,d] z[...]                        (DVE, bf16)
  bb += uv ; c = softmax_j(bb)                               (fp32)
Iteration 1 uses c = 0.1 exactly. Final squash + cross-core s-sum on host.
Matmuls run in float32r (TF32-like 1-pass) via AP bitcast; W is streamed
from HBM each phase; the uv-consume path is bf16 on DVE.

Per-core SBUF layouts (p^ = p % 128 on partitions, t = p//128 in 0..15):
  xt  [128, t16, d8, b32]        ws [128, t16, d8, j10, o16]
  wz  [j10, 128=(d*16+o), t16, p128]
"""
import numpy as np
import ml_dtypes
from functools import lru_cache

import concourse.bacc as bacc
import concourse.mybir as mybir
from concourse import tile
from concourse.bass_utils import run_bass_kernel_spmd

F32 = mybir.dt.float32
F32R = mybir.dt.float32r
BF16 = mybir.dt.bfloat16
AX = mybir.AxisListType
ALU = mybir.AluOpType
ACTF = mybir.ActivationFunctionType

B, J, P, D, O = 32, 10, 16384, 8, 16
NCORES = 8
PL = P // NCORES          # 2048
T = PL // 128             # 16 tiles of 128 p's
TG = 4                    # t-group size in z-phase
JO = J * O                # 160


def _emit(nc, n_cores):
    xt = nc.dram_tensor("xt", [128, T, D, B], F32R, kind="ExternalInput")
    xb = nc.dram_tensor("xb", [128, T, D, B], BF16, kind="ExternalInput")
    ws = nc.dram_tensor("ws", [128, T, D, J, O], F32R, kind="ExternalInput")
    wz = nc.dram_tensor("wz", [J, 128, T, 128], F32R, kind="ExternalInput")
    vz = nc.dram_tensor("vz", [128, J, D * B], F32R, kind="ExternalInput")
    s3p = nc.dram_tensor("s3p", [B, JO], F32, kind="ExternalOutput")

    with tile.TileContext(nc) as tc:
        with (
            tc.tile_pool(name="per", bufs=1) as per,        # persistent
            tc.tile_pool(name="wsst", bufs=3) as wsst,      # ws stream
            tc.tile_pool(name="wzst", bufs=3) as wzst,      # wz stream
            tc.tile_pool(name="yp", bufs=2) as yp,
            tc.tile_pool(name="zc", bufs=2) as zc,          # z consume bufs
            tc.tile_pool(name="small", bufs=2) as small,
            tc.tile_pool(name="sps", bufs=2, space="PSUM") as sps,
            tc.tile_pool(name="zps", bufs=2, space="PSUM") as zps,
            tc.tile_pool(name="dram", bufs=2, space="DRAM") as dramp,
        ):
            # warmup collective first: absorbs ncfw's first-collective
            # barrier (~40us) under the iter-0 compute. Contents junk.
            wu_in = dramp.tile([B, 16], F32)
            wu_out = dramp.tile([B, 16], F32)
            wu_sb = small.tile([B, 16], F32)
            nc.gpsimd.memset(wu_sb[:], 0.0)
            nc.sync.dma_start(wu_in[:], wu_sb[:])
            nc.gpsimd.collective_compute(
                "AllReduce", ALU.add,
                replica_groups=[list(range(n_cores))],
                ins=[wu_in[:].opt()], outs=[wu_out[:].opt()],
            )

            x_sb = per.tile([128, T, D, B], F32R)
            nc.sync.dma_start(x_sb[:], xt[:, :, :, :])
            xb_sb = per.tile([128, T, D, B], BF16)
            nc.sync.dma_start(xb_sb[:], xb[:, :, :, :])
            # block-diagonal v holder: rows (d*16+o), cols per j (d*32+b).
            vblk = per.tile([128, J, D * B], F32R)
            nc.sync.dma_start(vblk[:], vz[:, :, :])   # zeros (memset can't f32r)
            bb = per.tile([128, T, J, B], F32)      # routing logits
            e_sb = per.tile([128, T, J, B], F32)    # exp(bb)
            c_sb = per.tile([128, T, J, B], F32R)    # softmax coeffs
            se = per.tile([128, T, B], F32)         # sum_j exp
            rec = per.tile([128, T, B], F32)        # 1/sum

            for it in range(3):
                # ---------------- s-phase ----------------
                s_ps = sps.tile([B, 256], F32)
                if it > 0:
                    # softmax over j: c = exp(bb) / sum_j exp(bb)
                    nc.scalar.activation(e_sb[:], bb[:], ACTF.Exp)
                    nc.vector.tensor_reduce(
                        se[:, :, :, None],
                        e_sb.rearrange("p t j b -> p t b j"),
                        AX.X, ALU.add,
                    )
                    nc.vector.reciprocal(rec[:], se[:])
                    nc.gpsimd.tensor_mul(
                        c_sb[:], e_sb[:],
                        rec[:, :, None, :].broadcast_to([128, T, J, B]),
                    )
                for t in range(T):
                    wst = wsst.tile([128, D, J, O], F32R)
                    nc.sync.dma_start(wst[:], ws[:, t, :, :, :])
                    if it == 0:
                        # c == 0.1 exactly: lhsT = x, scale folded into copy.
                        # N padded 160->256 (reads run into the next d's
                        # region; junk lands in psum cols 160..255, ignored)
                        # to hit fp32r's 1-cycle/row regime; the last chunk
                        # can't overrun the tile so it stays N=160.
                        for d in range(D):
                            rhs = wst.rearrange("p d j o -> p (d j o)")
                            if d == D - 1:  # next-d overrun not possible
                                rhs = rhs[:, d * JO:(d + 1) * JO]
                            else:
                                rhs = rhs[:, d * JO:d * JO + 256]
                            nc.tensor.matmul(
                                s_ps[:, 0:rhs.shape[-1]],
                                x_sb[:, t, d, :],
                                rhs,
                                start=(t == 0 and d == 0),
                                stop=(t == T - 1 and d == D - 1),
                            )
                    else:
                        y_t = yp.tile([128, J, D, B], F32R)
                        # y = c * x, broadcast ops run at 1x -> split the
                        # work between DVE and GpSimd by t parity
                        eng = nc.vector if t % 2 == 0 else nc.gpsimd
                        eng.tensor_mul(
                            y_t[:],
                            c_sb[:, t, :, None, :].broadcast_to([128, J, D, B]),
                            x_sb[:, t, None, :, :].broadcast_to([128, J, D, B]),
                        )
                        for j in range(J):
                            for d in range(D):
                                # single accumulation group per psum bank
                                nc.tensor.matmul(
                                    s_ps[:, j * O:(j + 1) * O],
                                    y_t[:, j, d, :],
                                    wst[:, d, j, :],
                                    start=(t == 0 and j == 0 and d == 0),
                                    stop=(t == T - 1 and j == J - 1 and d == D - 1),
                                )
                s_sb = small.tile([B, JO], F32)
                nc.scalar.activation(s_sb[:], s_ps[:, 0:JO], ACTF.Copy,
                                     scale=0.1 if it == 0 else 1.0)
                if it == 2:
                    nc.sync.dma_start(s3p[:, :], s_sb[:])
                    break

                # ---------------- AllReduce s ----------------
                cc_in = dramp.tile([B, JO], F32)
                cc_out = dramp.tile([B, JO], F32)
                nc.sync.dma_start(cc_in[:], s_sb[:])
                nc.gpsimd.collective_compute(
                    "AllReduce", ALU.add,
                    replica_groups=[list(range(n_cores))],
                    ins=[cc_in[:].opt()], outs=[cc_out[:].opt()],
                )
                s_f = small.tile([B, JO], F32)
                nc.sync.dma_start(s_f[:], cc_out[:])

                # ---------------- squash -> v ----------------
                t2 = small.tile([B, JO], F32)
                nc.vector.tensor_mul(t2[:], s_f[:], s_f[:])
                sq = small.tile([B, J], F32)
                nc.vector.tensor_reduce(
                    sq[:, :, None], t2.rearrange("b (j o) -> b j o", j=J),
                    AX.X, ALU.add)
                r_ = small.tile([B, J], F32)
                nc.scalar.activation(r_[:], sq[:], ACTF.Sqrt)
                den = small.tile([B, J], F32)
                # den = (sq + 1) * r
                nc.vector.scalar_tensor_tensor(
                    den[:], sq[:], 1.0, r_[:], ALU.add, ALU.mult)
                rc2 = small.tile([B, J], F32)
                nc.vector.reciprocal(rc2[:], den[:])
                f_ = small.tile([B, J], F32)
                nc.vector.tensor_mul(f_[:], sq[:], rc2[:])
                v_sb = small.tile([B, J, O], F32R)
                nc.vector.tensor_mul(
                    v_sb[:], s_f.rearrange("b (j o) -> b j o", j=J),
                    f_[:, :, None].broadcast_to([B, J, O]))
                # bounce v through DRAM, then scatter transposed copies into
                # the block-diagonal slots (DMA is exempt from the 32-aligned
                # partition-start rule engine ops have)
                v_dr = dramp.tile([B, J, O], F32R)
                nc.sync.dma_start(v_dr[:], v_sb[:])
                for j in range(J):
                    for d in range(D):
                        nc.sync.dma_start(
                            vblk[d * O:(d + 1) * O, j, d * B:(d + 1) * B],
                            v_dr[:, j, :].rearrange("b o -> o b"))

                # ---------------- z / uv phase ----------------
                for j in range(J):
                    wzs = wzst.tile([128, T, 128], F32R)
                    nc.sync.dma_start(wzs[:], wz[j, :, :, :])
                    for tg in range(T // TG):
                        z_ps = zps.tile([128, TG, D * B], F32)
                        for t4 in range(TG):
                            # two 1KB outputs share each 2KB psum bank ->
                            # pair them into one group per bank
                            nc.tensor.matmul(
                                z_ps[:, t4, :], wzs[:, tg * TG + t4, :],
                                vblk[:, j, :],
                                start=(t4 % 2 == 0), stop=(t4 % 2 == 1))
                        ztmp = zc.tile([128, TG * D * B], BF16)
                        nc.scalar.copy(
                            ztmp[:], z_ps.rearrange("p t db -> p (t db)"))
                        tmp2 = zc.tile([128, TG * D * B], BF16)
                        nc.vector.tensor_mul(
                            tmp2[:], ztmp[:],
                            xb_sb[:, tg * TG:(tg + 1) * TG, :, :]
                            .rearrange("p t d b -> p (t d b)"))
                        t2v = tmp2.rearrange("p (t d b) -> p t d b", t=TG, d=D)
                        u1 = zc.tile([128, TG, 4, B], BF16)
                        nc.vector.tensor_add(
                            u1[:], t2v[:, :, 0:4, :], t2v[:, :, 4:8, :])
                        u2 = zc.tile([128, TG, 2, B], BF16)
                        nc.vector.tensor_add(
                            u2[:], u1[:, :, 0:2, :], u1[:, :, 2:4, :])
                        bb_sl = bb[:, tg * TG:(tg + 1) * TG, j, :]
                        if it == 0:
                            nc.vector.tensor_add(
                                bb_sl, u2[:, :, 0, :], u2[:, :, 1, :])
                        else:
                            uv = zc.tile([128, TG, B], F32)
                            nc.vector.tensor_add(
                                uv[:], u2[:, :, 0, :], u2[:, :, 1, :])
                            nc.vector.tensor_add(bb_sl, bb_sl, uv[:])
    return nc


@lru_cache(maxsize=2)
def _build(n_cores):
    nc = bacc.Bacc("TRN2", target_bir_lowering=False, debug=False,
                   num_devices=n_cores)
    _emit(nc, n_cores)
    nc.compile()
    return nc


def _prep_inputs(x, W):
    """Host-side shard + relayout. Returns list of per-core input dicts."""
    x = np.asarray(x, dtype=np.float32)
    W = np.asarray(W, dtype=np.float32)
    in_maps = []
    for c in range(NCORES):
        xc = x[:, c * PL:(c + 1) * PL, :]              # (B, PL, D)
        Wc = W[:, c * PL:(c + 1) * PL, :, :]           # (J, PL, D, O)
        xr = np.ascontiguousarray(
            xc.reshape(B, T, 128, D).transpose(2, 1, 3, 0))        # [128,T,D,B]
        wsr = np.ascontiguousarray(
            Wc.reshape(J, T, 128, D, O).transpose(2, 1, 3, 0, 4))  # [128,T,D,J,O]
        wzr = np.ascontiguousarray(
            Wc.reshape(J, T, 128, D, O).transpose(0, 3, 4, 1, 2)   # j,d,o,t,p
            .reshape(J, 128, T, 128))                              # [J,(d,o),T,p]
        in_maps.append({"xt": xr, "xb": xr.astype(ml_dtypes.bfloat16),
                        "ws": wsr, "wz": wzr,
                        "vz": np.zeros((128, J, D * B), np.float32)})
    return in_maps


def _squash_np(s):
    sq = np.sum(s * s, axis=-1, keepdims=True)
    return s * (sq / ((1.0 + sq) * np.sqrt(sq)))


def kernel(x, W):
    nc = _build(NCORES)
    in_maps = _prep_inputs(x, W)
    res = run_bass_kernel_spmd(nc, in_maps, list(range(NCORES)))
    s3 = np.zeros((B, JO), np.float64)
    for r in res.results:
        s3 += r["s3p"].astype(np.float64)
    v = _squash_np(s3.reshape(B, J, O))
    return v.astype(np.float32)



# revision 28
# speedup vs baseline: 1.1551x; 1.1551x over previous
"""DigitCaps dynamic-routing kernel for 8 Trainium2 NeuronCores.

Problem: x(32,16384,8) f32, W(10,16384,8,16) f32 -> v(32,10,16) f32
  u_hat[b,j,p,o] = sum_d x[b,p,d] W[j,p,d,o]   (never materialized)
  3 routing iterations (softmax over j, weighted sums over p).

Shard P=16384 over 8 cores (P_loc=2048, T=16 tiles of 128).
All matmuls bf16 (1 cyc/row on PE); W fully RESIDENT in SBUF in both
layouts (ws for s-phase, wz for z-phase; 10.5MB bf16 total) so HBM is
touched once. s-phase matmuls (K=p128, M=b32, N=o16) are packed 4-way
with PE column tiling: j -> col-group j//3 (tile_position=(0,32*(j//3))),
psum region [32*(j//3)+b, 16*(j%3)+o]. z-phase matmuls as in the
baseline (K=(d,o)=128, M=p128, N=(d,b)=256, block-diagonal v rhs) but
the block-diagonal vblk is built with 8 fat DMAs (one per d, 4-dim APs
into a J->12 padded tile) instead of 160 element scatters. z-consume
multiplies PSUM directly on DVE/GpSimd (alternating by j) and reduces
over d with tensor_reduce; softmax's 1/sum is folded into e so
y = e*x uses the bf16 x tile. Next-iteration softmax/y/s-matmuls are
pipelined tg-wise against the z-phase with the s-matmuls lagging one
tg behind the z-matmuls to keep the in-order PE queue from stalling.
"""
import numpy as np
import ml_dtypes
from functools import lru_cache

import concourse.bacc as bacc
import concourse.mybir as mybir
from concourse import tile
from concourse.bass_utils import run_bass_kernel_spmd

F32 = mybir.dt.float32
BF16 = mybir.dt.bfloat16
AX = mybir.AxisListType
ALU = mybir.AluOpType
ACTF = mybir.ActivationFunctionType

B, J, P, D, O = 32, 10, 16384, 8, 16
NCORES = 8
PL = P // NCORES          # 2048
T = PL // 128             # 16 tiles of 128 p's
TG = 4                    # t-group size in z-phase
NTG = T // TG             # 4
GCNT = (3, 3, 3, 1)       # j's per col-group: group jm holds j = 3*jm+k
SPC = 48                  # s psum col count: 3 blocks x O
# vblk/wz slot s=4k+jm holds logical j=3jm+k; (jm=3,k>0) slots are dead
SLOTS = [(s, 3 * (s % 4) + s // 4) for s in range(12)
         if s % 4 < 3 or s // 4 == 0]


def _emit(nc, n_cores):
    xb = nc.dram_tensor("xb", [128, T, D, B], BF16, kind="ExternalInput")
    xt = nc.dram_tensor("xt", [128, T, D, B], F32, kind="ExternalInput")
    ws = nc.dram_tensor("ws", [128, T, D, J, O], BF16, kind="ExternalInput")
    wz = nc.dram_tensor("wz", [128, 12, T, 128], BF16, kind="ExternalInput")
    vz = nc.dram_tensor("vz", [128, 12, 256], BF16, kind="ExternalInput")
    eye = nc.dram_tensor("eye", [128, 128], F32, kind="ExternalInput")
    s3p = nc.dram_tensor("s3p", [128, SPC], F32, kind="ExternalOutput")

    with tile.TileContext(nc) as tc:
        with (
            tc.tile_pool(name="per", bufs=1) as per,        # persistent
            tc.tile_pool(name="yp", bufs=2) as yp,          # y tiles (per tg)
            tc.tile_pool(name="zc", bufs=3) as zc,          # z consume tmps
            tc.tile_pool(name="uvp", bufs=2) as uvp,
            tc.tile_pool(name="small", bufs=2) as small,
            tc.tile_pool(name="sps", bufs=2, space="PSUM") as sps,
            tc.tile_pool(name="zps", bufs=2, space="PSUM") as zps,
            tc.tile_pool(name="tps", bufs=1, space="PSUM") as tps,
            tc.tile_pool(name="dram", bufs=2, space="DRAM") as dramp,
        ):
            # warmup collective: absorbs ncfw's first-collective barrier
            # under the it-0 compute. Contents junk.
            wu_in = dramp.tile([B, 16], F32)
            wu_out = dramp.tile([B, 16], F32)
            wu_sb = small.tile([B, 16], F32)
            nc.gpsimd.memset(wu_sb[:], 0.0)
            nc.gpsimd.dma_start(wu_in[:], wu_sb[:])
            nc.gpsimd.collective_compute(
                "AllReduce", ALU.add,
                replica_groups=[list(range(n_cores))],
                ins=[wu_in[:].opt()], outs=[wu_out[:].opt()],
            )

            # ---------------- persistent SBUF state ----------------
            xb_sb = per.tile([128, T, D, B], BF16)     # 8KB/part
            xt_sb = per.tile([128, T, D, B], F32)      # 16KB
            ws_sb = per.tile([128, T, D, J, O], BF16)  # 40KB
            wz_sb = per.tile([128, 12, T, 128], BF16)  # 48KB, slot-ordered j
            vblk = per.tile([128, 12, 256], BF16)      # 6KB, j padded to 12
            bb = per.tile([128, T, J, B], F32)         # 20KB routing logits
            e_sb = per.tile([128, T, J, B], BF16)      # 10KB exp(bb)*rec
            se = per.tile([128, T, B], F32)            # 2KB
            rec = per.tile([128, T, B], BF16)          # 1KB
            eye_sb = per.tile([128, 128], F32)         # 64KB identity

            nc.sync.dma_start(xb_sb[:], xb[:, :, :, :])
            nc.scalar.dma_start(xt_sb[:], xt[:, :, :, :])
            nc.gpsimd.dma_start(vblk[:], vz[:, :, :])
            nc.gpsimd.dma_start(eye_sb[:], eye[:, :])
            # W loads: chunked so they land on parallel DMA queues and
            # so it-0 s-matmuls can start on early t chunks.
            for t in range(T):
                nc.sync.dma_start(ws_sb[:, t], ws[:, t, :, :, :])
            for s, _ in SLOTS:
                nc.scalar.dma_start(wz_sb[:, s], wz[:, s, :, :])

            def s_matmuls0(s_ps):
                """it-0 s-phase: c uniform so stationary = xb and the
                rhs spans a whole col-group (N=16*GCNT[jm])."""
                for t in range(T):
                    for d in range(D):
                        for jm in range(4):
                            cnt = GCNT[jm]
                            nc.tensor.matmul(
                                s_ps[32 * jm:32 * jm + 32, 0:16 * cnt],
                                xb_sb[:, t, d, :],
                                ws_sb.rearrange(
                                    "p t d j o -> p t d (j o)")[
                                    :, t, d, 48 * jm:48 * jm + 16 * cnt],
                                start=(t == 0 and d == 0 and jm == 0),
                                stop=(t == T - 1 and d == D - 1 and jm == 3),
                                tile_position=(0, 32 * jm),
                            )

            def s_matmuls_iter(s_ps):
                """it>0 s-phase, d-outer: y_d = e (*) xb[:,:,d,:] (one
                broadcast input only), then matmuls over (t, j) with
                stationary y_d slices. Single closed accumulation
                group; no other PE group may interleave."""
                for d in range(D):
                    y_d = yp.tile([128, T, J, B], BF16)
                    eng = nc.vector if d % 2 == 0 else nc.gpsimd
                    eng.tensor_mul(
                        y_d[:], e_sb[:],
                        xb_sb[:, :, d, None, :].broadcast_to(
                            [128, T, J, B]))
                    for t in range(T):
                        for j in range(J):
                            jm, k = j // 3, j % 3
                            nc.tensor.matmul(
                                s_ps[32 * jm:32 * jm + 32,
                                     16 * k:16 * k + 16],
                                y_d[:, t, j, :],
                                ws_sb[:, t, d, j, :],
                                start=(d == 0 and t == 0 and j == 0),
                                stop=(d == D - 1 and t == T - 1
                                      and j == J - 1),
                                tile_position=(0, 32 * jm),
                            )

            def s_finish(it, s_ps):
                """psum -> s_sb (scaled for it0), AllReduce, squash -> v,
                scatter v into the block-diagonal vblk. For it==2 just
                writes the partial sums out."""
                # dead region (jm=3, cols 16:48) is never written by MMs;
                # zero it so AR/squash see finite junk.
                nc.vector.memset(s_ps[96:128, 16:SPC], 0.0)
                s_sb = small.tile([128, SPC], F32)
                nc.scalar.activation(s_sb[:], s_ps[:, 0:SPC], ACTF.Copy,
                                     scale=0.1 if it == 0 else 1.0)
                if it == 2:
                    nc.sync.dma_start(s3p[:, :], s_sb[:])
                    return
                cc_in = dramp.tile([128, SPC], F32)
                cc_out = dramp.tile([128, SPC], F32)
                nc.sync.dma_start(cc_in[:], s_sb[:])
                nc.gpsimd.collective_compute(
                    "AllReduce", ALU.add,
                    replica_groups=[list(range(n_cores))],
                    ins=[cc_in[:].opt()], outs=[cc_out[:].opt()],
                )
                s_f = small.tile([128, SPC], F32)
                nc.sync.dma_start(s_f[:], cc_out[:])
                # squash: v = s * sq/((1+sq)*sqrt(sq)) per (row, k-block)
                t2 = small.tile([128, SPC], F32)
                nc.vector.tensor_mul(t2[:], s_f[:], s_f[:])
                sq = small.tile([128, 3], F32)
                nc.vector.tensor_reduce(
                    sq[:, :, None], t2.rearrange("p (k o) -> p k o", k=3),
                    AX.X, ALU.add)
                r_ = small.tile([128, 3], F32)
                nc.scalar.activation(r_[:], sq[:], ACTF.Sqrt)
                den = small.tile([128, 3], F32)
                nc.vector.scalar_tensor_tensor(
                    den[:], sq[:], 1.0, r_[:], ALU.add, ALU.mult)
                rc2 = small.tile([128, 3], F32)
                nc.vector.reciprocal(rc2[:], den[:])
                f_ = small.tile([128, 3], F32)
                nc.vector.tensor_mul(f_[:], sq[:], rc2[:])
                v_sb = small.tile([128, 3, O], F32)
                nc.vector.tensor_mul(
                    v_sb[:], s_f.rearrange("p (k o) -> p k o", k=3),
                    f_[:, :, None].broadcast_to([128, 3, O]))
                # vblk slot j'=4k+jm holds logical j=3jm+k. PE-transpose
                # each k-block of v ([128=(jm,b), 16=o] -> [16=o,
                # 128=(jm,b)]), stage k-major in SBUF, then one clean
                # 3-dim DMA per d writes all 12 slots of a diagonal
                # block ((k,jm) merges: k-stride 128 = 4*32).
                vt_ps = tps.tile([128, 3, 128], F32)
                for k in range(3):
                    nc.tensor.transpose(
                        vt_ps[0:O, k, :], v_sb[:, k, :], eye_sb[:])
                v2a = small.tile([128, 3, 128], BF16)
                nc.scalar.activation(v2a[0:O], vt_ps[0:O], ACTF.Copy)
                v2v = v2a.rearrange("o k (jm b) -> o (k jm) b", jm=4)
                for d in range(D):
                    eng = nc.sync if d % 2 == 0 else nc.scalar
                    eng.dma_start(
                        vblk[16 * d:16 * d + 16, :, 32 * d:32 * d + 32],
                        v2v[0:O])

            def softmax_e(tg):
                """exp(bb) -> e, fold 1/sum_j into e, for tg's tiles."""
                ts = slice(tg * TG, (tg + 1) * TG)
                nc.scalar.activation(e_sb[:, ts], bb[:, ts], ACTF.Exp)
                nc.vector.tensor_reduce(
                    se[:, ts, :, None],
                    e_sb.rearrange("p t j b -> p t b j")[:, ts],
                    AX.X, ALU.add)
                with nc.allow_low_precision(reason="softmax weights are bf16"):
                    nc.vector.reciprocal(rec[:, ts], se[:, ts])
                nc.gpsimd.tensor_mul(
                    e_sb[:, ts], e_sb[:, ts],
                    rec[:, ts, None, :].broadcast_to([128, TG, J, B]))

            def z_phase_pipelined(it, s_ps_next):
                """z-phase of iteration `it` with the softmax/y of
                iteration it+1 pipelined tg-wise; the s-matmuls are
                emitted afterwards as one closed accumulation group
                (interleaving open PE accumulation groups crashes the
                walrus backend)."""
                for tg in range(NTG):
                    ts = slice(tg * TG, (tg + 1) * TG)
                    for s, j in SLOTS:
                        z_ps = zps.tile([128, TG, D * B], F32)
                        for t4 in range(TG):
                            nc.tensor.matmul(
                                z_ps[:, t4, :],
                                wz_sb[:, s, tg * TG + t4, :],
                                vblk[:, s, :],
                                start=(t4 % 2 == 0), stop=(t4 % 2 == 1))
                        # consume: GpSimd can't read PSUM, so alternate:
                        # even j -> DVE mul-from-psum + d-reduce;
                        # odd j -> Scalar psum->bf16 copy, GpSimd mul +
                        # tree-add over d.
                        if j % 2 == 0:
                            tmp = zc.tile([128, TG * D * B], BF16)
                            nc.vector.tensor_mul(
                                tmp[:], z_ps.rearrange("p t db -> p (t db)"),
                                xt_sb[:, ts].rearrange(
                                    "p t d b -> p (t d b)"))
                            tv = tmp.rearrange(
                                "p (t d b) -> p t b d", t=TG, d=D)
                            if it == 0:
                                nc.vector.tensor_reduce(
                                    bb[:, ts, j, :, None], tv, AX.X, ALU.add)
                            else:
                                uv = uvp.tile([128, TG, B], F32)
                                nc.vector.tensor_reduce(
                                    uv[:, :, :, None], tv, AX.X, ALU.add)
                                nc.vector.tensor_add(
                                    bb[:, ts, j, :], bb[:, ts, j, :], uv[:])
                        else:
                            ztmp = zc.tile([128, TG * D * B], BF16)
                            nc.scalar.copy(
                                ztmp[:], z_ps.rearrange("p t db -> p (t db)"))
                            tmp = zc.tile([128, TG * D * B], BF16)
                            nc.gpsimd.tensor_mul(
                                tmp[:], ztmp[:],
                                xb_sb[:, ts].rearrange(
                                    "p t d b -> p (t d b)"))
                            tv = tmp.rearrange(
                                "p (t d b) -> p t b d", t=TG, d=D)
                            u1 = zc.tile([128, TG, B, 4], BF16)
                            nc.gpsimd.tensor_add(
                                u1[:], tv[:, :, :, 0:4], tv[:, :, :, 4:8])
                            u2 = zc.tile([128, TG, B, 2], BF16)
                            nc.gpsimd.tensor_add(
                                u2[:], u1[:, :, :, 0:2], u1[:, :, :, 2:4])
                            if it == 0:
                                nc.gpsimd.tensor_add(
                                    bb[:, ts, j, :],
                                    u2[:, :, :, 0], u2[:, :, :, 1])
                            else:
                                uv = uvp.tile([128, TG, B], F32)
                                nc.gpsimd.tensor_add(
                                    uv[:], u2[:, :, :, 0], u2[:, :, :, 1])
                                nc.gpsimd.tensor_add(
                                    bb[:, ts, j, :], bb[:, ts, j, :], uv[:])
                    # all j done for this tg -> next-iter softmax
                    softmax_e(tg)
                s_matmuls_iter(s_ps_next)

            # ---------------- iteration 0 ----------------
            s_ps = sps.tile([128, SPC], F32)
            s_matmuls0(s_ps)
            s_finish(0, s_ps)

            # ---------------- iterations 1, 2 ----------------
            for it in range(2):
                s_ps = sps.tile([128, SPC], F32)
                z_phase_pipelined(it, s_ps)
                s_finish(it + 1, s_ps)
    return nc


@lru_cache(maxsize=2)
def _build(n_cores):
    nc = bacc.Bacc("TRN2", target_bir_lowering=False, debug=False,
                   num_devices=n_cores)
    _emit(nc, n_cores)
    nc.compile()
    return nc


def _prep_inputs(x, W):
    """Host-side shard + relayout. Returns list of per-core input dicts."""
    x = np.asarray(x, dtype=np.float32)
    W = np.asarray(W, dtype=np.float32)
    in_maps = []
    for c in range(NCORES):
        xc = x[:, c * PL:(c + 1) * PL, :]              # (B, PL, D)
        Wc = W[:, c * PL:(c + 1) * PL, :, :]           # (J, PL, D, O)
        xr = np.ascontiguousarray(
            xc.reshape(B, T, 128, D).transpose(2, 1, 3, 0))        # [128,T,D,B]
        wsr = np.ascontiguousarray(
            Wc.reshape(J, T, 128, D, O).transpose(2, 1, 3, 0, 4))  # [128,T,D,J,O]
        wzr = np.ascontiguousarray(
            Wc.reshape(J, T, 128, D, O).transpose(3, 4, 0, 1, 2)   # d,o,j,t,p
            .reshape(128, J, T, 128))                              # [(d,o),J,T,p]
        wzs = np.zeros((128, 12, T, 128), np.float32)
        for s, j in SLOTS:
            wzs[:, s] = wzr[:, j]
        in_maps.append({
            "xb": xr.astype(ml_dtypes.bfloat16),
            "xt": xr,
            "ws": wsr.astype(ml_dtypes.bfloat16),
            "wz": wzs.astype(ml_dtypes.bfloat16),
            "vz": np.zeros((128, 12, 256), ml_dtypes.bfloat16),
            "eye": np.eye(128, dtype=np.float32),
        })
    return in_maps


def _squash_np(s):
    sq = np.sum(s * s, axis=-1, keepdims=True)
    return s * (sq / ((1.0 + sq) * np.sqrt(sq)))


def _unpack_s(s3p_sum):
    """[128,48] partial-sum layout -> s[b, j, o]."""
    s = np.zeros((B, J, O), np.float64)
    for j in range(J):
        jm, k = j // 3, j % 3
        s[:, j, :] = s3p_sum[32 * jm:32 * jm + 32, 16 * k:16 * k + 16]
    return s


def kernel(x, W):
    nc = _build(NCORES)
    in_maps = _prep_inputs(x, W)
    res = run_bass_kernel_spmd(nc, in_maps, list(range(NCORES)))
    s3 = np.zeros((128, SPC), np.float64)
    for r in res.results:
        s3 += r["s3p"].astype(np.float64)
    v = _squash_np(_unpack_s(s3))
    return v.astype(np.float32)


# revision 37
# speedup vs baseline: 1.1602x; 1.0044x over previous
"""DigitCaps dynamic-routing kernel for 8 Trainium2 NeuronCores.

Problem: x(32,16384,8) f32, W(10,16384,8,16) f32 -> v(32,10,16) f32
  u_hat[b,j,p,o] = sum_d x[b,p,d] W[j,p,d,o]   (never materialized)
  3 routing iterations (softmax over j, weighted sums over p).

Shard P=16384 over 8 cores (P_loc=2048, T=16 tiles of 128).
All matmuls bf16 (1 cyc/row on PE); W fully RESIDENT in SBUF in both
layouts (ws for s-phase, wz for z-phase; 10.5MB bf16 total) so HBM is
touched once. s-phase matmuls (K=p128, M=b32, N=o16) are packed 4-way
with PE column tiling: j -> col-group j//3 (tile_position=(0,32*(j//3))),
psum region [32*(j//3)+b, 16*(j%3)+o]. z-phase matmuls as in the
baseline (K=(d,o)=128, M=p128, N=(d,b)=256, block-diagonal v rhs) but
the block-diagonal vblk is built with 8 fat DMAs (one per d, 4-dim APs
into a J->12 padded tile) instead of 160 element scatters. z-consume
multiplies PSUM directly on DVE/GpSimd (alternating by j) and reduces
over d with tensor_reduce; softmax's 1/sum is folded into e so
y = e*x uses the bf16 x tile. Next-iteration softmax/y/s-matmuls are
pipelined tg-wise against the z-phase with the s-matmuls lagging one
tg behind the z-matmuls to keep the in-order PE queue from stalling.
"""
import numpy as np
import ml_dtypes
from functools import lru_cache

import concourse.bacc as bacc
import concourse.mybir as mybir
from concourse import tile
from concourse.bass_utils import run_bass_kernel_spmd

F32 = mybir.dt.float32
BF16 = mybir.dt.bfloat16
AX = mybir.AxisListType
ALU = mybir.AluOpType
ACTF = mybir.ActivationFunctionType

B, J, P, D, O = 32, 10, 16384, 8, 16
NCORES = 8
PL = P // NCORES          # 2048
T = PL // 128             # 16 tiles of 128 p's
TG = 4                    # t-group size in z-phase
NTG = T // TG             # 4
GCNT = (3, 3, 3, 1)       # j's per col-group: group jm holds j = 3*jm+k
SPC = 48                  # s psum col count: 3 blocks x O
# vblk/wz slot s=4k+jm holds logical j=3jm+k; (jm=3,k>0) slots are dead
SLOTS = [(s, 3 * (s % 4) + s // 4) for s in range(12)
         if s % 4 < 3 or s // 4 == 0]


def _emit(nc, n_cores):
    xb = nc.dram_tensor("xb", [128, T, D, B], BF16, kind="ExternalInput")
    xt = nc.dram_tensor("xt", [128, T, D, B], F32, kind="ExternalInput")
    ws = nc.dram_tensor("ws", [128, T, D, J, O], BF16, kind="ExternalInput")
    wz = nc.dram_tensor("wz", [128, 10, T, 128], BF16, kind="ExternalInput")
    vz = nc.dram_tensor("vz", [128, 12, 256], BF16, kind="ExternalInput")
    eye = nc.dram_tensor("eye", [128, 128], F32, kind="ExternalInput")
    s3p = nc.dram_tensor("s3p", [128, SPC], F32, kind="ExternalOutput")

    with tile.TileContext(nc) as tc:
        with (
            tc.tile_pool(name="per", bufs=1) as per,        # persistent
            tc.tile_pool(name="yp", bufs=2) as yp,          # y tiles (per tg)
            tc.tile_pool(name="zc", bufs=3) as zc,          # z consume tmps
            tc.tile_pool(name="uvp", bufs=2) as uvp,
            tc.tile_pool(name="small", bufs=2) as small,
            tc.tile_pool(name="sps", bufs=2, space="PSUM") as sps,
            tc.tile_pool(name="zps", bufs=2, space="PSUM") as zps,
            tc.tile_pool(name="tps", bufs=1, space="PSUM") as tps,
            tc.tile_pool(name="dram", bufs=2, space="DRAM") as dramp,
        ):
            # warmup collective: absorbs ncfw's first-collective barrier
            # under the it-0 compute. Contents junk.
            wu_in = dramp.tile([B, 16], F32)
            wu_out = dramp.tile([B, 16], F32)
            wu_sb = small.tile([B, 16], F32)
            nc.gpsimd.memset(wu_sb[:], 0.0)
            nc.gpsimd.dma_start(wu_in[:], wu_sb[:])
            nc.gpsimd.collective_compute(
                "AllReduce", ALU.add,
                replica_groups=[list(range(n_cores))],
                ins=[wu_in[:].opt()], outs=[wu_out[:].opt()],
            )

            # ---------------- persistent SBUF state ----------------
            xb_sb = per.tile([128, T, D, B], BF16)     # 8KB/part
            xt_sb = per.tile([128, T, D, B], F32)      # 16KB
            ws_sb = per.tile([128, T, D, J, O], BF16)  # 40KB
            wz_sb = per.tile([128, 10, T, 128], BF16)  # 40KB, compact slots
            vblk = per.tile([128, 12, 256], BF16)      # 6KB, j padded to 12
            bb = per.tile([128, T, J, B], F32)         # 20KB routing logits
            e_sb = per.tile([128, T, J, B], F32)       # 20KB exp(bb)*rec
            se = per.tile([128, T, B], F32)            # 2KB
            rec = per.tile([128, T, B], F32)           # 2KB
            eye_sb = per.tile([128, 128], F32)         # 64KB identity

            nc.sync.dma_start(xb_sb[:], xb[:, :, :, :])
            nc.scalar.dma_start(xt_sb[:], xt[:, :, :, :])
            nc.gpsimd.dma_start(vblk[:], vz[:, :, :])
            nc.gpsimd.dma_start(eye_sb[:], eye[:, :])
            # W loads: chunked so they land on parallel DMA queues and
            # so it-0 s-matmuls can start on early t chunks.
            for t in range(T):
                nc.sync.dma_start(ws_sb[:, t], ws[:, t, :, :, :])
            for i in range(len(SLOTS)):
                nc.scalar.dma_start(wz_sb[:, i], wz[:, i, :, :])

            def s_matmuls0(s_ps):
                """it-0 s-phase: c uniform so stationary = xb and the
                rhs spans a whole col-group (N=16*GCNT[jm])."""
                for t in range(T):
                    for d in range(D):
                        for jm in range(4):
                            cnt = GCNT[jm]
                            nc.tensor.matmul(
                                s_ps[32 * jm:32 * jm + 32, 0:16 * cnt],
                                xb_sb[:, t, d, :],
                                ws_sb.rearrange(
                                    "p t d j o -> p t d (j o)")[
                                    :, t, d, 48 * jm:48 * jm + 16 * cnt],
                                start=(t == 0 and d == 0 and jm == 0),
                                stop=(t == T - 1 and d == D - 1 and jm == 3),
                                tile_position=(0, 32 * jm),
                            )

            def s_matmuls_iter(s_ps):
                """it>0 s-phase, d-outer: y_d = e (*) xb[:,:,d,:] (one
                broadcast input only), then matmuls over (t, j) with
                stationary y_d slices. Single closed accumulation
                group; no other PE group may interleave."""
                for d in range(D):
                    y_d = yp.tile([128, T, J, B], BF16)
                    # split t-range DVE/GpSimd (~rate-balanced 10/6)
                    nc.vector.tensor_mul(
                        y_d[:, 0:10], e_sb[:, 0:10],
                        xt_sb[:, 0:10, d, None, :].broadcast_to(
                            [128, 10, J, B]))
                    nc.gpsimd.tensor_mul(
                        y_d[:, 10:T], e_sb[:, 10:T],
                        xt_sb[:, 10:T, d, None, :].broadcast_to(
                            [128, T - 10, J, B]))
                    for t in range(T):
                        for j in range(J):
                            jm, k = j // 3, j % 3
                            nc.tensor.matmul(
                                s_ps[32 * jm:32 * jm + 32,
                                     16 * k:16 * k + 16],
                                y_d[:, t, j, :],
                                ws_sb[:, t, d, j, :],
                                start=(d == 0 and t == 0 and j == 0),
                                stop=(d == D - 1 and t == T - 1
                                      and j == J - 1),
                                tile_position=(0, 32 * jm),
                            )

            def s_finish(it, s_ps):
                """psum -> s_sb (scaled for it0), AllReduce, squash -> v,
                scatter v into the block-diagonal vblk. For it==2 just
                writes the partial sums out."""
                # dead region (jm=3, cols 16:48) is never written by MMs;
                # zero it so AR/squash see finite junk.
                nc.vector.memset(s_ps[96:128, 16:SPC], 0.0)
                s_sb = small.tile([128, SPC], F32)
                nc.scalar.activation(s_sb[:], s_ps[:, 0:SPC], ACTF.Copy,
                                     scale=0.1 if it == 0 else 1.0)
                if it == 2:
                    nc.sync.dma_start(s3p[:, :], s_sb[:])
                    return
                # AllReduce split by k-block so z-matmuls for slots 0:8
                # (k=0,1) start while the k=2 AllReduce is in flight.
                vt_ps = tps.tile([128, 3, 128], F32)
                v2a = small.tile([128, 3, 128], BF16)
                for (c0, c1, k0, k1, s0, s1) in (
                        (0, 32, 0, 2, 0, 8), (32, SPC, 2, 3, 8, 12)):
                    nk = k1 - k0
                    cc_in = dramp.tile([128, c1 - c0], F32)
                    cc_out = dramp.tile([128, c1 - c0], F32)
                    nc.sync.dma_start(cc_in[:], s_sb[:, c0:c1])
                    nc.gpsimd.collective_compute(
                        "AllReduce", ALU.add,
                        replica_groups=[list(range(n_cores))],
                        ins=[cc_in[:].opt()], outs=[cc_out[:].opt()],
                    )
                    s_f = small.tile([128, c1 - c0], F32)
                    nc.sync.dma_start(s_f[:], cc_out[:])
                    # squash: v = s * sq/((1+sq)*sqrt(sq)) per (row, k)
                    t2 = small.tile([128, c1 - c0], F32)
                    nc.vector.tensor_mul(t2[:], s_f[:], s_f[:])
                    sq = small.tile([128, nk], F32)
                    nc.vector.tensor_reduce(
                        sq[:, :, None],
                        t2.rearrange("p (k o) -> p k o", k=nk),
                        AX.X, ALU.add)
                    r_ = small.tile([128, nk], F32)
                    nc.scalar.activation(r_[:], sq[:], ACTF.Sqrt)
                    den = small.tile([128, nk], F32)
                    nc.vector.scalar_tensor_tensor(
                        den[:], sq[:], 1.0, r_[:], ALU.add, ALU.mult)
                    rc2 = small.tile([128, nk], F32)
                    nc.vector.reciprocal(rc2[:], den[:])
                    f_ = small.tile([128, nk], F32)
                    nc.vector.tensor_mul(f_[:], sq[:], rc2[:])
                    v_sb = small.tile([128, nk, O], F32)
                    nc.vector.tensor_mul(
                        v_sb[:], s_f.rearrange("p (k o) -> p k o", k=nk),
                        f_[:, :, None].broadcast_to([128, nk, O]))
                    # vblk slot j'=4k+jm holds logical j=3jm+k.
                    # PE-transpose each k-block of v ([128=(jm,b), 16=o]
                    # -> [16=o, 128=(jm,b)]), stage k-major, then one
                    # 3-dim DMA per d writes this batch's slots of a
                    # diagonal block ((k,jm) merges: stride 128 = 4*32).
                    for ki in range(nk):
                        nc.tensor.transpose(
                            vt_ps[0:O, k0 + ki, :], v_sb[:, ki, :],
                            eye_sb[:])
                    nc.scalar.activation(v2a[0:O, k0:k1],
                                         vt_ps[0:O, k0:k1], ACTF.Copy)
                    v2v = v2a.rearrange("o k (jm b) -> o (k jm) b", jm=4)
                    for d in range(D):
                        eng = nc.sync if d % 2 == 0 else nc.scalar
                        eng.dma_start(
                            vblk[16 * d:16 * d + 16, s0:s1,
                                 32 * d:32 * d + 32],
                            v2v[0:O, s0:s1])

            def softmax_e(tg):
                """exp(bb) -> e, fold 1/sum_j into e, for tg's tiles."""
                ts = slice(tg * TG, (tg + 1) * TG)
                nc.scalar.activation(e_sb[:, ts], bb[:, ts], ACTF.Exp)
                nc.vector.tensor_reduce(
                    se[:, ts, :, None],
                    e_sb.rearrange("p t j b -> p t b j")[:, ts],
                    AX.X, ALU.add)
                nc.vector.reciprocal(rec[:, ts], se[:, ts])
                nc.gpsimd.tensor_mul(
                    e_sb[:, ts], e_sb[:, ts],
                    rec[:, ts, None, :].broadcast_to([128, TG, J, B]))

            def z_phase_pipelined(it, s_ps_next):
                """z-phase of iteration `it` with the softmax/y of
                iteration it+1 pipelined tg-wise; the s-matmuls are
                emitted afterwards as one closed accumulation group
                (interleaving open PE accumulation groups crashes the
                walrus backend)."""
                for tg in range(NTG):
                    ts = slice(tg * TG, (tg + 1) * TG)
                    for i, (s, j) in enumerate(SLOTS):
                        z_ps = zps.tile([128, TG, D * B], F32)
                        for t4 in range(TG):
                            nc.tensor.matmul(
                                z_ps[:, t4, :],
                                wz_sb[:, i, tg * TG + t4, :],
                                vblk[:, s, :],
                                start=(t4 % 2 == 0), stop=(t4 % 2 == 1))
                        # consume: DVE does all muls (GpSimd's mul uop is
                        # 2.4 cyc/elem and it can't read PSUM); Scalar
                        # pre-copies PSUM->bf16 for odd slots; GpSimd
                        # does the d-sum as contiguous half-tree adds.
                        tmp = zc.tile([128, TG * D * B], BF16)
                        if i % 2 == 0:
                            nc.vector.tensor_mul(
                                tmp[:], z_ps.rearrange("p t db -> p (t db)"),
                                xt_sb[:, ts].rearrange(
                                    "p t d b -> p (t d b)"))
                        else:
                            ztmp = zc.tile([128, TG * D * B], BF16)
                            nc.scalar.copy(
                                ztmp[:], z_ps.rearrange("p t db -> p (t db)"))
                            nc.vector.tensor_mul(
                                tmp[:], ztmp[:],
                                xb_sb[:, ts].rearrange(
                                    "p t d b -> p (t d b)"))
                        tv = tmp.rearrange(
                            "p (t d b) -> p t d b", t=TG, d=D)
                        u1 = zc.tile([128, TG, 4, B], BF16)
                        nc.gpsimd.tensor_add(
                            u1[:], tv[:, :, 0:4, :], tv[:, :, 4:8, :])
                        u2 = zc.tile([128, TG, 2, B], BF16)
                        nc.gpsimd.tensor_add(
                            u2[:], u1[:, :, 0:2, :], u1[:, :, 2:4, :])
                        if it == 0:
                            nc.gpsimd.tensor_add(
                                bb[:, ts, j, :],
                                u2[:, :, 0, :], u2[:, :, 1, :])
                        else:
                            uv = uvp.tile([128, TG, B], F32)
                            nc.gpsimd.tensor_add(
                                uv[:], u2[:, :, 0, :], u2[:, :, 1, :])
                            nc.gpsimd.tensor_add(
                                bb[:, ts, j, :], bb[:, ts, j, :], uv[:])
                    # all j done for this tg -> next-iter softmax
                    softmax_e(tg)
                s_matmuls_iter(s_ps_next)

            # ---------------- iteration 0 ----------------
            s_ps = sps.tile([128, SPC], F32)
            s_matmuls0(s_ps)
            s_finish(0, s_ps)

            # ---------------- iterations 1, 2 ----------------
            for it in range(2):
                s_ps = sps.tile([128, SPC], F32)
                z_phase_pipelined(it, s_ps)
                s_finish(it + 1, s_ps)
    return nc


@lru_cache(maxsize=2)
def _build(n_cores):
    nc = bacc.Bacc("TRN2", target_bir_lowering=False, debug=False,
                   num_devices=n_cores)
    _emit(nc, n_cores)
    nc.compile()
    return nc


def _prep_inputs(x, W):
    """Host-side shard + relayout. Returns list of per-core input dicts."""
    x = np.asarray(x, dtype=np.float32)
    W = np.asarray(W, dtype=np.float32)
    in_maps = []
    for c in range(NCORES):
        xc = x[:, c * PL:(c + 1) * PL, :]              # (B, PL, D)
        Wc = W[:, c * PL:(c + 1) * PL, :, :]           # (J, PL, D, O)
        xr = np.ascontiguousarray(
            xc.reshape(B, T, 128, D).transpose(2, 1, 3, 0))        # [128,T,D,B]
        wsr = np.ascontiguousarray(
            Wc.reshape(J, T, 128, D, O).transpose(2, 1, 3, 0, 4))  # [128,T,D,J,O]
        wzr = np.ascontiguousarray(
            Wc.reshape(J, T, 128, D, O).transpose(3, 4, 0, 1, 2)   # d,o,j,t,p
            .reshape(128, J, T, 128))                              # [(d,o),J,T,p]
        wzs = np.zeros((128, 10, T, 128), np.float32)
        for i, (s, j) in enumerate(SLOTS):
            wzs[:, i] = wzr[:, j]
        in_maps.append({
            "xb": xr.astype(ml_dtypes.bfloat16),
            "xt": xr,
            "ws": wsr.astype(ml_dtypes.bfloat16),
            "wz": wzs.astype(ml_dtypes.bfloat16),
            "vz": np.zeros((128, 12, 256), ml_dtypes.bfloat16),
            "eye": np.eye(128, dtype=np.float32),
        })
    return in_maps


def _squash_np(s):
    sq = np.sum(s * s, axis=-1, keepdims=True)
    return s * (sq / ((1.0 + sq) * np.sqrt(sq)))


def _unpack_s(s3p_sum):
    """[128,48] partial-sum layout -> s[b, j, o]."""
    s = np.zeros((B, J, O), np.float64)
    for j in range(J):
        jm, k = j // 3, j % 3
        s[:, j, :] = s3p_sum[32 * jm:32 * jm + 32, 16 * k:16 * k + 16]
    return s


def kernel(x, W):
    nc = _build(NCORES)
    in_maps = _prep_inputs(x, W)
    res = run_bass_kernel_spmd(nc, in_maps, list(range(NCORES)))
    s3 = np.zeros((128, SPC), np.float64)
    for r in res.results:
        s3 += r["s3p"].astype(np.float64)
    v = _squash_np(_unpack_s(s3))
    return v.astype(np.float32)


# revision 39
# speedup vs baseline: 1.2112x; 1.0439x over previous
"""DigitCaps dynamic-routing kernel for 8 Trainium2 NeuronCores.

Problem: x(32,16384,8) f32, W(10,16384,8,16) f32 -> v(32,10,16) f32
  u_hat[b,j,p,o] = sum_d x[b,p,d] W[j,p,d,o]   (never materialized)
  3 routing iterations (softmax over j, weighted sums over p).

Shard P=16384 over 8 cores (P_loc=2048, T=16 tiles of 128).
All matmuls bf16 (1 cyc/row on PE); W fully RESIDENT in SBUF in both
layouts (ws for s-phase, wz for z-phase; 10.5MB bf16 total) so HBM is
touched once. s-phase matmuls (K=p128, M=b32, N=o16) are packed 4-way
with PE column tiling: j -> col-group j//3 (tile_position=(0,32*(j//3))),
psum region [32*(j//3)+b, 16*(j%3)+o]. z-phase matmuls as in the
baseline (K=(d,o)=128, M=p128, N=(d,b)=256, block-diagonal v rhs) but
the block-diagonal vblk is built with 8 fat DMAs (one per d, 4-dim APs
into a J->12 padded tile) instead of 160 element scatters. z-consume
multiplies PSUM directly on DVE/GpSimd (alternating by j) and reduces
over d with tensor_reduce; softmax's 1/sum is folded into e so
y = e*x uses the bf16 x tile. Next-iteration softmax/y/s-matmuls are
pipelined tg-wise against the z-phase with the s-matmuls lagging one
tg behind the z-matmuls to keep the in-order PE queue from stalling.
"""
import numpy as np
import ml_dtypes
from functools import lru_cache

import concourse.bacc as bacc
import concourse.mybir as mybir
from concourse import tile
from concourse.bass_utils import run_bass_kernel_spmd

F32 = mybir.dt.float32
BF16 = mybir.dt.bfloat16
AX = mybir.AxisListType
ALU = mybir.AluOpType
ACTF = mybir.ActivationFunctionType

B, J, P, D, O = 32, 10, 16384, 8, 16
NCORES = 8
PL = P // NCORES          # 2048
T = PL // 128             # 16 tiles of 128 p's
TG = 4                    # t-group size in z-phase
NTG = T // TG             # 4
GCNT = (3, 3, 3, 1)       # j's per col-group: group jm holds j = 3*jm+k
SPC = 48                  # s psum col count: 3 blocks x O
# vblk/wz slot s=4k+jm holds logical j=3jm+k; (jm=3,k>0) slots are dead
SLOTS = [(s, 3 * (s % 4) + s // 4) for s in range(12)
         if s % 4 < 3 or s // 4 == 0]


def _emit(nc, n_cores):
    xb = nc.dram_tensor("xb", [128, T, D, B], BF16, kind="ExternalInput")
    xt = nc.dram_tensor("xt", [128, T, D, B], F32, kind="ExternalInput")
    ws = nc.dram_tensor("ws", [128, T, D, J, O], BF16, kind="ExternalInput")
    wz = nc.dram_tensor("wz", [128, 10, T, 128], BF16, kind="ExternalInput")
    vz = nc.dram_tensor("vz", [128, 12, 256], BF16, kind="ExternalInput")
    eye = nc.dram_tensor("eye", [128, 128], F32, kind="ExternalInput")
    s3p = nc.dram_tensor("s3p", [128, SPC], F32, kind="ExternalOutput")

    with tile.TileContext(nc) as tc:
        with (
            tc.tile_pool(name="per", bufs=1) as per,        # persistent
            tc.tile_pool(name="yp", bufs=2) as yp,          # y tiles (per tg)
            tc.tile_pool(name="zc", bufs=3) as zc,          # z consume tmps
            tc.tile_pool(name="uvp", bufs=2) as uvp,
            tc.tile_pool(name="small", bufs=2) as small,
            tc.tile_pool(name="sps", bufs=2, space="PSUM") as sps,
            tc.tile_pool(name="zps", bufs=2, space="PSUM") as zps,
            tc.tile_pool(name="tps", bufs=1, space="PSUM") as tps,
            tc.tile_pool(name="dram", bufs=2, space="DRAM") as dramp,
        ):
            # warmup collective: absorbs ncfw's first-collective barrier
            # under the it-0 compute. Contents junk.
            wu_in = dramp.tile([B, 16], F32)
            wu_out = dramp.tile([B, 16], F32)
            wu_sb = small.tile([B, 16], F32)
            nc.gpsimd.memset(wu_sb[:], 0.0)
            nc.gpsimd.dma_start(wu_in[:], wu_sb[:])
            nc.gpsimd.collective_compute(
                "AllReduce", ALU.add,
                replica_groups=[list(range(n_cores))],
                ins=[wu_in[:].opt()], outs=[wu_out[:].opt()],
            )

            # ---------------- persistent SBUF state ----------------
            xb_sb = per.tile([128, T, D, B], BF16)     # 8KB/part
            xt_sb = per.tile([128, T, D, B], F32)      # 16KB
            ws_sb = per.tile([128, T, D, J, O], BF16)  # 40KB
            wz_sb = per.tile([128, 10, T, 128], BF16)  # 40KB, compact slots
            vblk = per.tile([128, 12, 256], BF16)      # 6KB, j padded to 12
            bb = per.tile([128, T, J, B], F32)         # 20KB routing logits
            e_sb = per.tile([128, T, J, B], F32)       # 20KB exp(bb)*rec
            se = per.tile([128, T, B], F32)            # 2KB
            rec = per.tile([128, T, B], F32)           # 2KB
            eye_sb = per.tile([128, 128], F32)         # 64KB identity

            nc.sync.dma_start(xb_sb[:], xb[:, :, :, :])
            nc.scalar.dma_start(xt_sb[:], xt[:, :, :, :])
            nc.gpsimd.dma_start(vblk[:], vz[:, :, :])
            nc.gpsimd.dma_start(eye_sb[:], eye[:, :])
            # W loads: chunked so they land on parallel DMA queues and
            # so it-0 s-matmuls can start on early t chunks.
            for t in range(T):
                nc.sync.dma_start(ws_sb[:, t], ws[:, t, :, :, :])
            for i in range(len(SLOTS)):
                nc.scalar.dma_start(wz_sb[:, i], wz[:, i, :, :])

            def s_matmuls0(s_ps):
                """it-0 s-phase: c uniform so stationary = xb and the
                rhs spans a whole col-group (N=16*GCNT[jm])."""
                for t in range(T):
                    for d in range(D):
                        for jm in range(4):
                            cnt = GCNT[jm]
                            nc.tensor.matmul(
                                s_ps[32 * jm:32 * jm + 32, 0:16 * cnt],
                                xb_sb[:, t, d, :],
                                ws_sb.rearrange(
                                    "p t d j o -> p t d (j o)")[
                                    :, t, d, 48 * jm:48 * jm + 16 * cnt],
                                # start clears THIS col-group's partition
                                # slice of the bank, so flag per group
                                start=(t == 0 and d == 0),
                                stop=(t == T - 1 and d == D - 1),
                                tile_position=(0, 32 * jm),
                                skip_group_check=True,
                            )

            def s_matmuls_iter(s_ps):
                """it>0 s-phase, d-outer: y_d = e (*) xb[:,:,d,:] (one
                broadcast input only), then matmuls over (t, j) with
                stationary y_d slices. Single closed accumulation
                group; no other PE group may interleave."""
                for d in range(D):
                    y_d = yp.tile([128, T, J, B], BF16)
                    # split t-range DVE/GpSimd (~rate-balanced 10/6)
                    nc.vector.tensor_mul(
                        y_d[:, 0:10], e_sb[:, 0:10],
                        xt_sb[:, 0:10, d, None, :].broadcast_to(
                            [128, 10, J, B]))
                    nc.gpsimd.tensor_mul(
                        y_d[:, 10:T], e_sb[:, 10:T],
                        xt_sb[:, 10:T, d, None, :].broadcast_to(
                            [128, T - 10, J, B]))
                    for t in range(T):
                        for j in range(J):
                            jm, k = j // 3, j % 3
                            nc.tensor.matmul(
                                s_ps[32 * jm:32 * jm + 32,
                                     16 * k:16 * k + 16],
                                y_d[:, t, j, :],
                                ws_sb[:, t, d, j, :],
                                # start/stop per col-group (start clears
                                # the group's whole partition slice)
                                start=(d == 0 and t == 0 and k == 0),
                                stop=(d == D - 1 and t == T - 1
                                      and (k == 2 or j == J - 1)),
                                tile_position=(0, 32 * jm),
                                skip_group_check=True,
                            )

            def s_finish(it, s_ps):
                """psum -> s_sb (scaled for it0), AllReduce, squash -> v,
                scatter v into the block-diagonal vblk. For it==2 just
                writes the partial sums out."""
                # dead region (jm=3, cols 16:48) is never written by MMs;
                # zero it so AR/squash see finite junk.
                nc.vector.memset(s_ps[96:128, 16:SPC], 0.0)
                s_sb = small.tile([128, SPC], F32)
                nc.scalar.activation(s_sb[:], s_ps[:, 0:SPC], ACTF.Copy,
                                     scale=0.1 if it == 0 else 1.0)
                if it == 2:
                    nc.sync.dma_start(s3p[:, :], s_sb[:])
                    return
                # AllReduce split by k-block so z-matmuls for slots 0:8
                # (k=0,1) start while the k=2 AllReduce is in flight.
                vt_ps = tps.tile([128, 3, 128], F32)
                v2a = small.tile([128, 3, 128], BF16)
                for (c0, c1, k0, k1, s0, s1) in (
                        (0, 32, 0, 2, 0, 8), (32, SPC, 2, 3, 8, 12)):
                    nk = k1 - k0
                    cc_in = dramp.tile([128, c1 - c0], F32)
                    cc_out = dramp.tile([128, c1 - c0], F32)
                    nc.sync.dma_start(cc_in[:], s_sb[:, c0:c1])
                    nc.gpsimd.collective_compute(
                        "AllReduce", ALU.add,
                        replica_groups=[list(range(n_cores))],
                        ins=[cc_in[:].opt()], outs=[cc_out[:].opt()],
                    )
                    s_f = small.tile([128, c1 - c0], F32)
                    nc.sync.dma_start(s_f[:], cc_out[:])
                    # squash: v = s * sq/((1+sq)*sqrt(sq)) per (row, k)
                    t2 = small.tile([128, c1 - c0], F32)
                    nc.vector.tensor_mul(t2[:], s_f[:], s_f[:])
                    sq = small.tile([128, nk], F32)
                    nc.vector.tensor_reduce(
                        sq[:, :, None],
                        t2.rearrange("p (k o) -> p k o", k=nk),
                        AX.X, ALU.add)
                    r_ = small.tile([128, nk], F32)
                    nc.scalar.activation(r_[:], sq[:], ACTF.Sqrt)
                    den = small.tile([128, nk], F32)
                    nc.vector.scalar_tensor_tensor(
                        den[:], sq[:], 1.0, r_[:], ALU.add, ALU.mult)
                    rc2 = small.tile([128, nk], F32)
                    nc.vector.reciprocal(rc2[:], den[:])
                    f_ = small.tile([128, nk], F32)
                    nc.vector.tensor_mul(f_[:], sq[:], rc2[:])
                    v_sb = small.tile([128, nk, O], F32)
                    nc.vector.tensor_mul(
                        v_sb[:], s_f.rearrange("p (k o) -> p k o", k=nk),
                        f_[:, :, None].broadcast_to([128, nk, O]))
                    # vblk slot j'=4k+jm holds logical j=3jm+k.
                    # PE-transpose each k-block of v ([128=(jm,b), 16=o]
                    # -> [16=o, 128=(jm,b)]), stage k-major, then one
                    # 3-dim DMA per d writes this batch's slots of a
                    # diagonal block ((k,jm) merges: stride 128 = 4*32).
                    for ki in range(nk):
                        nc.tensor.transpose(
                            vt_ps[0:O, k0 + ki, :], v_sb[:, ki, :],
                            eye_sb[:])
                    nc.scalar.activation(v2a[0:O, k0:k1],
                                         vt_ps[0:O, k0:k1], ACTF.Copy)
                    v2v = v2a.rearrange("o k (jm b) -> o (k jm) b", jm=4)
                    for d in range(D):
                        eng = nc.sync if d % 2 == 0 else nc.scalar
                        eng.dma_start(
                            vblk[16 * d:16 * d + 16, s0:s1,
                                 32 * d:32 * d + 32],
                            v2v[0:O, s0:s1])

            def softmax_e(tg):
                """exp(bb) -> e, fold 1/sum_j into e, for tg's tiles."""
                ts = slice(tg * TG, (tg + 1) * TG)
                nc.scalar.activation(e_sb[:, ts], bb[:, ts], ACTF.Exp)
                nc.vector.tensor_reduce(
                    se[:, ts, :, None],
                    e_sb.rearrange("p t j b -> p t b j")[:, ts],
                    AX.X, ALU.add)
                nc.vector.reciprocal(rec[:, ts], se[:, ts])
                nc.gpsimd.tensor_mul(
                    e_sb[:, ts], e_sb[:, ts],
                    rec[:, ts, None, :].broadcast_to([128, TG, J, B]))

            def z_phase_pipelined(it, s_ps_next):
                """z-phase of iteration `it` with the softmax/y of
                iteration it+1 pipelined tg-wise; the s-matmuls are
                emitted afterwards as one closed accumulation group
                (interleaving open PE accumulation groups crashes the
                walrus backend)."""
                for tg in range(NTG):
                    ts = slice(tg * TG, (tg + 1) * TG)
                    for i, (s, j) in enumerate(SLOTS):
                        z_ps = zps.tile([128, TG, D * B], F32)
                        for t4 in range(TG):
                            nc.tensor.matmul(
                                z_ps[:, t4, :],
                                wz_sb[:, i, tg * TG + t4, :],
                                vblk[:, s, :],
                                start=(t4 % 2 == 0), stop=(t4 % 2 == 1))
                        # consume: DVE does all muls (GpSimd's mul uop is
                        # 2.4 cyc/elem and it can't read PSUM); Scalar
                        # pre-copies PSUM->bf16 for odd slots; GpSimd
                        # does the d-sum as contiguous half-tree adds.
                        tmp = zc.tile([128, TG * D * B], BF16)
                        if i % 2 == 0:
                            nc.vector.tensor_mul(
                                tmp[:], z_ps.rearrange("p t db -> p (t db)"),
                                xt_sb[:, ts].rearrange(
                                    "p t d b -> p (t d b)"))
                        else:
                            ztmp = zc.tile([128, TG * D * B], BF16)
                            nc.scalar.copy(
                                ztmp[:], z_ps.rearrange("p t db -> p (t db)"))
                            nc.vector.tensor_mul(
                                tmp[:], ztmp[:],
                                xb_sb[:, ts].rearrange(
                                    "p t d b -> p (t d b)"))
                        tv = tmp.rearrange(
                            "p (t d b) -> p t d b", t=TG, d=D)
                        u1 = zc.tile([128, TG, 4, B], BF16)
                        nc.gpsimd.tensor_add(
                            u1[:], tv[:, :, 0:4, :], tv[:, :, 4:8, :])
                        u2 = zc.tile([128, TG, 2, B], BF16)
                        nc.gpsimd.tensor_add(
                            u2[:], u1[:, :, 0:2, :], u1[:, :, 2:4, :])
                        if it == 0:
                            nc.gpsimd.tensor_add(
                                bb[:, ts, j, :],
                                u2[:, :, 0, :], u2[:, :, 1, :])
                        else:
                            uv = uvp.tile([128, TG, B], F32)
                            nc.gpsimd.tensor_add(
                                uv[:], u2[:, :, 0, :], u2[:, :, 1, :])
                            nc.gpsimd.tensor_add(
                                bb[:, ts, j, :], bb[:, ts, j, :], uv[:])
                    # all j done for this tg -> next-iter softmax
                    softmax_e(tg)
                s_matmuls_iter(s_ps_next)

            # ---------------- iteration 0 ----------------
            s_ps = sps.tile([128, SPC], F32)
            s_matmuls0(s_ps)
            s_finish(0, s_ps)

            # ---------------- iterations 1, 2 ----------------
            for it in range(2):
                s_ps = sps.tile([128, SPC], F32)
                z_phase_pipelined(it, s_ps)
                s_finish(it + 1, s_ps)
    return nc


@lru_cache(maxsize=2)
def _build(n_cores):
    nc = bacc.Bacc("TRN2", target_bir_lowering=False, debug=False,
                   num_devices=n_cores)
    _emit(nc, n_cores)
    nc.compile()
    return nc


def _prep_inputs(x, W):
    """Host-side shard + relayout. Returns list of per-core input dicts."""
    x = np.asarray(x, dtype=np.float32)
    W = np.asarray(W, dtype=np.float32)
    in_maps = []
    for c in range(NCORES):
        xc = x[:, c * PL:(c + 1) * PL, :]              # (B, PL, D)
        Wc = W[:, c * PL:(c + 1) * PL, :, :]           # (J, PL, D, O)
        xr = np.ascontiguousarray(
            xc.reshape(B, T, 128, D).transpose(2, 1, 3, 0))        # [128,T,D,B]
        wsr = np.ascontiguousarray(
            Wc.reshape(J, T, 128, D, O).transpose(2, 1, 3, 0, 4))  # [128,T,D,J,O]
        wzr = np.ascontiguousarray(
            Wc.reshape(J, T, 128, D, O).transpose(3, 4, 0, 1, 2)   # d,o,j,t,p
            .reshape(128, J, T, 128))                              # [(d,o),J,T,p]
        wzs = np.zeros((128, 10, T, 128), np.float32)
        for i, (s, j) in enumerate(SLOTS):
            wzs[:, i] = wzr[:, j]
        in_maps.append({
            "xb": xr.astype(ml_dtypes.bfloat16),
            "xt": xr,
            "ws": wsr.astype(ml_dtypes.bfloat16),
            "wz": wzs.astype(ml_dtypes.bfloat16),
            "vz": np.zeros((128, 12, 256), ml_dtypes.bfloat16),
            "eye": np.eye(128, dtype=np.float32),
        })
    return in_maps


def _squash_np(s):
    sq = np.sum(s * s, axis=-1, keepdims=True)
    return s * (sq / ((1.0 + sq) * np.sqrt(sq)))


def _unpack_s(s3p_sum):
    """[128,48] partial-sum layout -> s[b, j, o]."""
    s = np.zeros((B, J, O), np.float64)
    for j in range(J):
        jm, k = j // 3, j % 3
        s[:, j, :] = s3p_sum[32 * jm:32 * jm + 32, 16 * k:16 * k + 16]
    return s


def kernel(x, W):
    nc = _build(NCORES)
    in_maps = _prep_inputs(x, W)
    res = run_bass_kernel_spmd(nc, in_maps, list(range(NCORES)))
    s3 = np.zeros((128, SPC), np.float64)
    for r in res.results:
        s3 += r["s3p"].astype(np.float64)
    v = _squash_np(_unpack_s(s3))
    return v.astype(np.float32)
